# revision 29
# baseline (speedup 1.0000x reference)
"""CP-decomposed 4D linear layer on 8 Trainium2 NeuronCores.

out[b, cls] = sum_r lam[r] * U4[cls,r] * sum_c U3[c,r] * sum_w U2[w,r] * sum_h U1[h,r] * x[b,c,w,h]

Strategy (data-parallel over batch, 16 b per core):
  - host precomputes G[r, w*32+h] = U2[w,r]*U1[h,r]  (64 x 1024, f32)
    and A[r, cls] = lam[r]*U4[cls,r]                  (64 x 1000, f32),
    casts x to bf16 laid out [b][p][k][f] with c = k*128 + p so each
    per-batch SBUF load is ONE fully-linear 1MB DMA (8KB partition
    lines; descriptors fan out across all 16 DMA engines), and
    pre-reorders U3 to [p][k][r] bf16 zero-padded to r=128
    (128 stationary columns enable PE fast weight load).
  - per b: PE contracts c (K=512 as 4 accumulated chunks of 128):
        t[r, f] = sum_c U3[c,r] * x[b,c,f]   -> PSUM [128, 1024] (bf16 matmul)
  - one fused DVE pass multiplies by G and reduces over f:
        z[r, b] = sum_f t[r,f]*G[r,f]
    (scalar_tensor_tensor with accum_out; NB tensor_tensor_reduce crashes
     trn2 HW even though CoreSim passes)
  - final PE matmul out[b,cls] = sum_r z[r,b]*A[r,cls] in two b-halves; the
    first half is emitted mid-loop so it overlaps the b-loop tail.
  - x streams on the sync-engine HWDGE queue; constants + output use the
    scalar-engine queue so x descriptors start flowing immediately.
"""

import numpy as np
import ml_dtypes

import concourse.bass as bass
import concourse.bacc as bacc
import concourse.mybir as mybir
import concourse.tile as tile
from concourse.bass_utils import run_bass_kernel_spmd

B, C, W, H, CLS, R = 128, 512, 32, 32, 1000, 64
WH = W * H          # 1024
N_CORES = 8
B_LOC = B // N_CORES  # 16
KC = C // 128         # 4 contraction chunks
BF16 = mybir.dt.bfloat16
F32 = mybir.dt.float32

_NC_CACHE = {}


def _build(reps=1, xbufs=6, use_ttr="stt", single_dma=True, use_scalar_q=True,
           pad_m=True, split_out=True, xq="full", fine_tail=True):
    MR = 128 if pad_m else R  # stationary columns; 128 enables FWL
    nc = bacc.Bacc()
    x = nc.declare_dram_parameter("x", [B_LOC, 128, KC, WH], BF16, isOutput=False)
    u3 = nc.declare_dram_parameter("u3", [128, KC, MR], BF16, isOutput=False)
    g = nc.declare_dram_parameter("g", [R, WH], F32, isOutput=False)
    a = nc.declare_dram_parameter("a", [R, CLS], F32, isOutput=False)
    out = nc.declare_dram_parameter("out", [B_LOC, CLS], F32, isOutput=True)

    cq = nc.scalar if use_scalar_q else nc.sync

    with tile.TileContext(nc) as tc:
        with (
            tc.tile_pool(name="const", bufs=1) as cpool,
            tc.tile_pool(name="xp", bufs=xbufs) as xpool,
            tc.tile_pool(name="tmp", bufs=2) as tpool,
            tc.tile_pool(name="ps", bufs=3, space="PSUM") as pspool,
            tc.tile_pool(name="psd", bufs=1, space="PSUM") as psdpool,
        ):
            u3s = cpool.tile([128, KC, MR], BF16)
            cq.dma_start(u3s[:], u3[:])
            gs = cpool.tile([R, WH], F32)
            cq.dma_start(gs[:], g[:])
            asb = cpool.tile([R, CLS], F32)
            cq.dma_start(asb[:], a[:])
            zbuf = cpool.tile([R, B_LOC], F32)

            def emit_out_half(lo, hi):
                # out[lo:hi, cls] = sum_r zbuf[r, lo:hi] * A[r, cls]
                n = hi - lo
                od = psdpool.tile([n, CLS], F32, tag="od")
                nc.tensor.matmul(
                    od[:, 0:512], zbuf[:, lo:hi], asb[:, 0:512],
                    start=True, stop=True,
                )
                nc.tensor.matmul(
                    od[:, 512:CLS], zbuf[:, lo:hi], asb[:, 512:CLS],
                    start=True, stop=True,
                )
                osb = cpool.tile([n, CLS], F32, tag=f"osb{lo}")
                nc.vector.tensor_copy(osb[:], od[:])
                cq.dma_start(out[lo:hi], osb[:])

            for rep in range(reps):
                for b in range(B_LOC):
                    xb = xpool.tile([128, KC, WH], BF16, tag="xb")
                    if xq == "full":
                        if fine_tail and b in (0, B_LOC - 1):
                            # first b: quarter loads so PE starts one k-chunk
                            # after the first bytes (PE is saturated, so its
                            # whole stream shifts earlier); last b: quarter
                            # loads so its matmuls trail the final bytes by
                            # one k-chunk, not a full 1MB
                            for k in range(KC):
                                nc.sync.dma_start(xb[:, k, :], x[b, :, k, :])
                        elif fine_tail and b in (1, B_LOC - 2):
                            nc.sync.dma_start(xb[:, 0:2, :], x[b, :, 0:2, :])
                            nc.sync.dma_start(xb[:, 2:4, :], x[b, :, 2:4, :])
                        else:
                            # one linear 1MB load per b (fewest triggers)
                            nc.sync.dma_start(xb[:], x[b])
                    elif single_dma:
                        # two half-loads (k 0-1, k 2-3) so the first matmuls
                        # start while the second half streams
                        h2q = nc.scalar if xq == "dual" else nc.sync
                        nc.sync.dma_start(xb[:, 0:2, :], x[b, :, 0:2, :])
                        h2q.dma_start(xb[:, 2:4, :], x[b, :, 2:4, :])
                    else:
                        for k in range(KC):
                            nc.sync.dma_start(xb[:, k, :], x[b, :, k, :])

                    tps = pspool.tile([MR, WH], F32, tag="tps")
                    for k in range(KC):
                        for n in range(2):
                            sl = bass.ts(n, 512)
                            nc.tensor.matmul(
                                tps[:, sl],
                                u3s[:, k, :],
                                xb[:, k, sl],
                                start=(k == 0),
                                stop=(k == KC - 1),
                            )

                    if use_ttr == "sttd":
                        # stt with stride-0 dummy out: only accum_out is kept,
                        # skipping the 256KB/b SBUF write-back
                        dummy = tpool.tile([R, 1], F32, tag="ttr")
                        nc.vector.scalar_tensor_tensor(
                            dummy.broadcast_to((R, WH)),
                            tps[0:R],
                            1.0,
                            gs[:],
                            mybir.AluOpType.mult,
                            mybir.AluOpType.mult,
                            accum_out=zbuf[:, b : b + 1],
                        )
                    elif use_ttr == "stt":
                        # fused multiply+reduce via SCALAR_TENSOR_TENSOR:
                        # out = (tps * 1.0) * gs ; accum_out = sum(out)
                        tmp = tpool.tile([R, WH], F32, tag="ttr")
                        nc.vector.scalar_tensor_tensor(
                            tmp[:],
                            tps[0:R],
                            1.0,
                            gs[:],
                            mybir.AluOpType.mult,
                            mybir.AluOpType.mult,
                            accum_out=zbuf[:, b : b + 1],
                        )
                    elif use_ttr:
                        # qr.py-style: out is a stride-0 dummy (only accum_out
                        # is kept) — saves the full-size SBUF write.
                        dummy = tpool.tile([R, 1], F32, tag="ttr")
                        nc.vector.tensor_tensor_reduce(
                            dummy.broadcast_to((R, WH)),
                            gs[:],
                            tps[:],
                            1.0,
                            0.0,
                            mybir.AluOpType.mult,
                            mybir.AluOpType.add,
                            zbuf[:, b : b + 1],
                        )
                    else:
                        tmp = tpool.tile([R, WH], F32, tag="ttr")
                        nc.vector.tensor_tensor(
                            tmp[:], tps[0:R], gs[:], mybir.AluOpType.mult
                        )
                        nc.vector.tensor_reduce(
                            zbuf[:, b : b + 1],
                            tmp[:],
                            mybir.AxisListType.X,
                            mybir.AluOpType.add,
                        )

                    if split_out and rep == reps - 1:
                        if fine_tail:
                            # output in 8/4/4 chunks: earlier chunks overlap
                            # the b-loop; the final chain covers only 4 rows
                            if b == 7:
                                emit_out_half(0, 8)
                            elif b == 11:
                                emit_out_half(8, 12)
                        elif b == B_LOC // 2 - 1:
                            emit_out_half(0, B_LOC // 2)

            # step D: out[b, cls] = sum_r zbuf[r, b] * A[r, cls]
            if split_out and fine_tail:
                emit_out_half(12, B_LOC)
            elif split_out:
                emit_out_half(B_LOC // 2, B_LOC)
            else:
                emit_out_half(0, B_LOC)

    nc.compile()
    return nc


def _get_nc():
    if "v3" not in _NC_CACHE:
        _NC_CACHE["v3"] = _build()
    return _NC_CACHE["v3"]


def _prep_inputs(x, U1, U2, U3, U4, lam):
    x = np.asarray(x, dtype=np.float32).reshape(B, KC, 128, WH)
    U1 = np.asarray(U1, dtype=np.float32)
    U2 = np.asarray(U2, dtype=np.float32)
    U3 = np.asarray(U3, dtype=np.float32)
    U4 = np.asarray(U4, dtype=np.float32)
    lam = np.asarray(lam, dtype=np.float32)

    # x [B, k, p, f] -> [B, p, k, f] bf16 (c = k*128 + p): one linear 1MB
    # DMA per (core, b) with 8KB partition lines.
    xh = np.empty((B, 128, KC, WH), dtype=ml_dtypes.bfloat16)
    xh[...] = x.transpose(0, 2, 1, 3)
    # U3 [(k p), r] -> [p, k, r] bf16, zero-padded r 64->128 (stationary
    # columns = 128 enables PE fast weight load; pad rows are ignored)
    u3h = np.zeros((128, KC, 128), dtype=ml_dtypes.bfloat16)
    u3h[:, :, :R] = U3.reshape(KC, 128, R).transpose(1, 0, 2)
    # G[r, w*32+h] = U2[w,r] * U1[h,r]
    G = np.ascontiguousarray(
        (U2.T[:, :, None] * U1.T[:, None, :]).reshape(R, WH).astype(np.float32)
    )
    # A[r, cls] = lam[r] * U4[cls, r]
    A = np.ascontiguousarray((U4 * lam[None, :]).T.astype(np.float32))

    in_maps = [
        {
            "x": np.ascontiguousarray(xh[i * B_LOC : (i + 1) * B_LOC]),
            "u3": u3h,
            "g": G,
            "a": A,
        }
        for i in range(N_CORES)
    ]
    return in_maps


def kernel(x, U1, U2, U3, U4, lam):
    in_maps = _prep_inputs(x, U1, U2, U3, U4, lam)
    nc = _get_nc()
    res = run_bass_kernel_spmd(nc, in_maps, list(range(N_CORES)))
    return np.concatenate([res.results[i]["out"] for i in range(N_CORES)], axis=0)


# revision 30
# speedup vs baseline: 1.0174x; 1.0174x over previous
"""CP-decomposed 4D linear layer on 8 Trainium2 NeuronCores.

out[b, cls] = sum_r lam[r] * U4[cls,r] * sum_c U3[c,r] * sum_w U2[w,r] * sum_h U1[h,r] * x[b,c,w,h]

Strategy (data-parallel over batch, 16 b per core):
  - host precomputes G[r, w*32+h] = U2[w,r]*U1[h,r]  (64 x 1024, f32)
    and A[r, cls] = lam[r]*U4[cls,r]                  (64 x 1000, f32),
    casts x to bf16 laid out [b][p][k][f] with c = k*128 + p so each
    per-batch SBUF load is ONE fully-linear 1MB DMA (8KB partition
    lines; descriptors fan out across all 16 DMA engines), and
    pre-reorders U3 to [p][k][r] bf16 zero-padded to r=128
    (128 stationary columns enable PE fast weight load).
  - per b: PE contracts c (K=512 as 4 accumulated chunks of 128):
        t[r, f] = sum_c U3[c,r] * x[b,c,f]   -> PSUM [128, 1024] (bf16 matmul)
  - one fused DVE pass multiplies by G and reduces over f:
        z[r, b] = sum_f t[r,f]*G[r,f]
    (scalar_tensor_tensor with accum_out; NB tensor_tensor_reduce crashes
     trn2 HW even though CoreSim passes)
  - final PE matmul out[b,cls] = sum_r z[r,b]*A[r,cls] in two b-halves; the
    first half is emitted mid-loop so it overlaps the b-loop tail.
  - x streams on the sync-engine HWDGE queue; constants + output use the
    scalar-engine queue so x descriptors start flowing immediately.
"""

import numpy as np
import ml_dtypes

import concourse.bass as bass
import concourse.bacc as bacc
import concourse.mybir as mybir
import concourse.tile as tile
from concourse.bass_utils import run_bass_kernel_spmd

B, C, W, H, CLS, R = 128, 512, 32, 32, 1000, 64
WH = W * H          # 1024
N_CORES = 8
B_LOC = B // N_CORES  # 16
KC = C // 128         # 4 contraction chunks
BF16 = mybir.dt.bfloat16
F32 = mybir.dt.float32

_NC_CACHE = {}


def _build(reps=1, xbufs=6, use_ttr="stt", single_dma=True, use_scalar_q=True,
           pad_m=True, split_out=True, xq="full", fine_tail=True):
    MR = 128 if pad_m else R  # stationary columns; 128 enables FWL
    nc = bacc.Bacc()
    x = nc.declare_dram_parameter("x", [B_LOC, 128, KC, WH], BF16, isOutput=False)
    u3 = nc.declare_dram_parameter("u3", [128, KC, MR], BF16, isOutput=False)
    g = nc.declare_dram_parameter("g", [R, WH], F32, isOutput=False)
    a = nc.declare_dram_parameter("a", [R, CLS], F32, isOutput=False)
    out = nc.declare_dram_parameter("out", [B_LOC, CLS], F32, isOutput=True)

    cq = nc.scalar if use_scalar_q else nc.sync

    with tile.TileContext(nc) as tc:
        with (
            tc.tile_pool(name="const", bufs=1) as cpool,
            tc.tile_pool(name="xp", bufs=xbufs) as xpool,
            tc.tile_pool(name="tmp", bufs=2) as tpool,
            tc.tile_pool(name="ps", bufs=3, space="PSUM") as pspool,
            tc.tile_pool(name="psd", bufs=1, space="PSUM") as psdpool,
        ):
            u3s = cpool.tile([128, KC, MR], BF16)
            cq.dma_start(u3s[:], u3[:])
            gs = cpool.tile([R, WH], F32)
            cq.dma_start(gs[:], g[:])
            asb = cpool.tile([R, CLS], F32)
            cq.dma_start(asb[:], a[:])
            zbuf = cpool.tile([R, B_LOC], F32)

            def emit_out_half(lo, hi):
                # out[lo:hi, cls] = sum_r zbuf[r, lo:hi] * A[r, cls]
                n = hi - lo
                od = psdpool.tile([n, CLS], F32, tag="od")
                nc.tensor.matmul(
                    od[:, 0:512], zbuf[:, lo:hi], asb[:, 0:512],
                    start=True, stop=True,
                )
                nc.tensor.matmul(
                    od[:, 512:CLS], zbuf[:, lo:hi], asb[:, 512:CLS],
                    start=True, stop=True,
                )
                osb = cpool.tile([n, CLS], F32, tag=f"osb{lo}")
                nc.vector.tensor_copy(osb[:], od[:])
                cq.dma_start(out[lo:hi], osb[:])

            for rep in range(reps):
                for b in range(B_LOC):
                    xb = xpool.tile([128, KC, WH], BF16, tag="xb")
                    if xq == "full":
                        if fine_tail and b == B_LOC - 1:
                            # last b: quarter loads so its matmuls trail the
                            # final bytes by one k-chunk, not a full 1MB
                            # (head-splitting b=0 was tried and is a net loss:
                            # extra triggers delay the bulk DMA stream, and
                            # DMA is the longer pole — PE's slack is at the
                            # start)
                            for k in range(KC):
                                nc.sync.dma_start(xb[:, k, :], x[b, :, k, :])
                        elif fine_tail and b == B_LOC - 2:
                            nc.sync.dma_start(xb[:, 0:2, :], x[b, :, 0:2, :])
                            nc.sync.dma_start(xb[:, 2:4, :], x[b, :, 2:4, :])
                        else:
                            # one linear 1MB load per b (fewest triggers)
                            nc.sync.dma_start(xb[:], x[b])
                    elif single_dma:
                        # two half-loads (k 0-1, k 2-3) so the first matmuls
                        # start while the second half streams
                        h2q = nc.scalar if xq == "dual" else nc.sync
                        nc.sync.dma_start(xb[:, 0:2, :], x[b, :, 0:2, :])
                        h2q.dma_start(xb[:, 2:4, :], x[b, :, 2:4, :])
                    else:
                        for k in range(KC):
                            nc.sync.dma_start(xb[:, k, :], x[b, :, k, :])

                    tps = pspool.tile([MR, WH], F32, tag="tps")
                    for k in range(KC):
                        for n in range(2):
                            sl = bass.ts(n, 512)
                            nc.tensor.matmul(
                                tps[:, sl],
                                u3s[:, k, :],
                                xb[:, k, sl],
                                start=(k == 0),
                                stop=(k == KC - 1),
                            )

                    if use_ttr == "sttd":
                        # stt with stride-0 dummy out: only accum_out is kept,
                        # skipping the 256KB/b SBUF write-back
                        dummy = tpool.tile([R, 1], F32, tag="ttr")
                        nc.vector.scalar_tensor_tensor(
                            dummy.broadcast_to((R, WH)),
                            tps[0:R],
                            1.0,
                            gs[:],
                            mybir.AluOpType.mult,
                            mybir.AluOpType.mult,
                            accum_out=zbuf[:, b : b + 1],
                        )
                    elif use_ttr == "stt":
                        # fused multiply+reduce via SCALAR_TENSOR_TENSOR:
                        # out = (tps * 1.0) * gs ; accum_out = sum(out)
                        tmp = tpool.tile([R, WH], F32, tag="ttr")
                        nc.vector.scalar_tensor_tensor(
                            tmp[:],
                            tps[0:R],
                            1.0,
                            gs[:],
                            mybir.AluOpType.mult,
                            mybir.AluOpType.mult,
                            accum_out=zbuf[:, b : b + 1],
                        )
                    elif use_ttr:
                        # qr.py-style: out is a stride-0 dummy (only accum_out
                        # is kept) — saves the full-size SBUF write.
                        dummy = tpool.tile([R, 1], F32, tag="ttr")
                        nc.vector.tensor_tensor_reduce(
                            dummy.broadcast_to((R, WH)),
                            gs[:],
                            tps[:],
                            1.0,
                            0.0,
                            mybir.AluOpType.mult,
                            mybir.AluOpType.add,
                            zbuf[:, b : b + 1],
                        )
                    else:
                        tmp = tpool.tile([R, WH], F32, tag="ttr")
                        nc.vector.tensor_tensor(
                            tmp[:], tps[0:R], gs[:], mybir.AluOpType.mult
                        )
                        nc.vector.tensor_reduce(
                            zbuf[:, b : b + 1],
                            tmp[:],
                            mybir.AxisListType.X,
                            mybir.AluOpType.add,
                        )

                    if split_out and rep == reps - 1:
                        if fine_tail:
                            # output in 8/4/4 chunks: earlier chunks overlap
                            # the b-loop; the final chain covers only 4 rows
                            if b == 7:
                                emit_out_half(0, 8)
                            elif b == 11:
                                emit_out_half(8, 12)
                        elif b == B_LOC // 2 - 1:
                            emit_out_half(0, B_LOC // 2)

            # step D: out[b, cls] = sum_r zbuf[r, b] * A[r, cls]
            if split_out and fine_tail:
                emit_out_half(12, B_LOC)
            elif split_out:
                emit_out_half(B_LOC // 2, B_LOC)
            else:
                emit_out_half(0, B_LOC)

    nc.compile()
    return nc


def _get_nc():
    if "v3" not in _NC_CACHE:
        _NC_CACHE["v3"] = _build()
    return _NC_CACHE["v3"]


def _prep_inputs(x, U1, U2, U3, U4, lam):
    x = np.asarray(x, dtype=np.float32).reshape(B, KC, 128, WH)
    U1 = np.asarray(U1, dtype=np.float32)
    U2 = np.asarray(U2, dtype=np.float32)
    U3 = np.asarray(U3, dtype=np.float32)
    U4 = np.asarray(U4, dtype=np.float32)
    lam = np.asarray(lam, dtype=np.float32)

    # x [B, k, p, f] -> [B, p, k, f] bf16 (c = k*128 + p): one linear 1MB
    # DMA per (core, b) with 8KB partition lines.
    xh = np.empty((B, 128, KC, WH), dtype=ml_dtypes.bfloat16)
    xh[...] = x.transpose(0, 2, 1, 3)
    # U3 [(k p), r] -> [p, k, r] bf16, zero-padded r 64->128 (stationary
    # columns = 128 enables PE fast weight load; pad rows are ignored)
    u3h = np.zeros((128, KC, 128), dtype=ml_dtypes.bfloat16)
    u3h[:, :, :R] = U3.reshape(KC, 128, R).transpose(1, 0, 2)
    # G[r, w*32+h] = U2[w,r] * U1[h,r]
    G = np.ascontiguousarray(
        (U2.T[:, :, None] * U1.T[:, None, :]).reshape(R, WH).astype(np.float32)
    )
    # A[r, cls] = lam[r] * U4[cls, r]
    A = np.ascontiguousarray((U4 * lam[None, :]).T.astype(np.float32))

    in_maps = [
        {
            "x": np.ascontiguousarray(xh[i * B_LOC : (i + 1) * B_LOC]),
            "u3": u3h,
            "g": G,
            "a": A,
        }
        for i in range(N_CORES)
    ]
    return in_maps


def kernel(x, U1, U2, U3, U4, lam):
    in_maps = _prep_inputs(x, U1, U2, U3, U4, lam)
    nc = _get_nc()
    res = run_bass_kernel_spmd(nc, in_maps, list(range(N_CORES)))
    return np.concatenate([res.results[i]["out"] for i in range(N_CORES)], axis=0)
